# revision 8
# baseline (speedup 1.0000x reference)
"""CWT latent attention kernel for 8 Trainium2 NeuronCores.

Sharding: core c = 2*b + g handles batch b and head-group g (8 of 16 heads).
Each core computes its heads' q/k/v, causal attention, and a partial output
projection (contracted over its heads' channels); the host sums the two
partials per batch.

Device layout notes:
- All matmul operands live "transposed" (contraction dim on partitions);
  activations are fed pre-transposed from the host, so no on-device
  transposes are needed anywhere.
- Attention computes scoresT = K^T-tile.T @ Q (shape [s, tq]); softmax skips
  the max-subtraction (scores are O(10), exp cannot overflow in fp32), the
  causal mask is a 0/1 multiply on diagonal tiles, and the denominator is a
  ones-vector matmul accumulated alongside the A@V matmul. The division is
  applied to the attention output tiles via a partition-broadcast of 1/den.
- RoPE is applied in [dh, t] layout as rot = x*C + swap(x*S'), where C/S'
  are host-precomputed [128, T] tables and swap exchanges adjacent partition
  pairs via two strided SBUF->SBUF DMAs.
- Matmuls run in float32r (TF32); inputs are TF32-rounded on the host, and
  every on-device tensor feeding a matmul is written with dtype float32r.
"""

import math
from dataclasses import dataclass

import numpy as np

import concourse.bass as bass
import concourse.mybir as mybir
import concourse.tile as tile
from concourse import bacc
from concourse.bass_utils import run_bass_kernel_spmd

F32 = mybir.dt.float32
F32R = mybir.dt.float32r
EXP = mybir.ActivationFunctionType.Exp
SQRT = mybir.ActivationFunctionType.Sqrt
MUL = mybir.AluOpType.mult
ADD = mybir.AluOpType.add

# problem constants
B, T, D = 4, 2048, 2048
H, DH = 16, 128
D_LAT, D_HUB = 512, 1024
EPS = 1e-6
G = 2               # head groups == cores per batch
HG = H // G         # heads per core
NCORES = 8
TQ = 512            # tq group width for attention


@dataclass
class Cfg:
    T: int = T
    D: int = D
    DHUB: int = D_HUB
    DLAT: int = D_LAT
    HG: int = HG
    TH: int = 2      # t-halves for XT residency in the q-projection

    @property
    def DC(self):  return self.D // 128        # xt chunks
    @property
    def HC(self):  return self.DHUB // 128     # hub chunks
    @property
    def LC(self):  return self.DLAT // 128     # latent chunks
    @property
    def ST(self):  return self.T // 128        # s tiles
    @property
    def NG(self):  return self.T // TQ         # tq groups
    @property
    def T4(self):  return self.T // 512        # 512-wide column chunks
    @property
    def THW(self): return self.T // self.TH    # t-half width
    @property
    def GD(self):  return self.HG * DH         # group channel width
    @property
    def EC(self):  return self.D // 512        # output e columns


def round_tf32(x: np.ndarray) -> np.ndarray:
    x = np.ascontiguousarray(x, np.float32)
    u = x.view(np.uint32)
    r = (u + 0x1000 + ((u >> 13) & 1)) & np.uint32(0xFFFFE000)
    return r.view(np.float32)


def input_specs(P: Cfg):
    return {
        "xt":      ([P.D, P.T], F32),
        "hubt":    ([P.DHUB, P.T], F32),
        "wqt":     ([P.D, P.GD], F32),
        "wdt":     ([P.DHUB, P.DLAT], F32),
        "wut_k":   ([P.DLAT, P.GD], F32),
        "wut_v":   ([P.DLAT, P.GD], F32),
        "wot":     ([P.GD, P.D], F32),
        "rope_c":  ([128, P.T], F32),
        "rope_sp": ([128, P.T], F32),
        "masks":   ([4, 128, TQ], F32),
        "ones_w":  ([128, 1], F32),
        "perm":    ([128, 128], F32),
    }


def _bcast_ap(row: bass.AP, p: int = 128) -> bass.AP:
    """[1, N] DRAM row -> partition-broadcast [p, N] read AP."""
    return bass.AP(tensor=row.tensor, offset=row.offset,
                   ap=[[0, p]] + [list(d) for d in row.ap[1:]])


def build_kernel(tc: tile.TileContext, outs: dict, ins: dict, P: Cfg):
    nc = tc.nc
    scale = 1.0 / math.sqrt(DH)
    out_p = outs["out_p"]

    with tc.tile_pool(name="tables", bufs=1) as tables, \
         tc.tile_pool(name="dram", bufs=1, space="DRAM") as dram, \
         tc.tile_pool(name="ckvpool", bufs=1) as ckvpool:

        rope_c = tables.tile([128, P.T], F32)
        nc.sync.dma_start(out=rope_c, in_=ins["rope_c"][:])
        rope_sp = tables.tile([128, P.T], F32)
        nc.sync.dma_start(out=rope_sp, in_=ins["rope_sp"][:])
        # masks dram is [4,128,TQ]; load each mask as a [128, TQ] tile
        masks_t = [tables.tile([128, TQ], F32, name=f"mask{r}", tag=f"mask{r}")
                   for r in range(4)]
        for r in range(4):
            nc.sync.dma_start(out=masks_t[r], in_=ins["masks"][r])
        ones_sb = tables.tile([128, 1], F32R)
        nc.sync.dma_start(out=ones_sb, in_=ins["ones_w"][:].bitcast(F32R))
        eps_sb = tables.tile([1, 1], F32)
        nc.vector.memset(eps_sb, EPS)
        perm_sb = tables.tile([128, 128], F32R)
        nc.sync.dma_start(out=perm_sb, in_=ins["perm"][:].bitcast(F32R))

        ckv = ckvpool.tile([128, P.LC, P.T], F32R)

        qspill = dram.tile([P.HG, 128, P.T], F32R)
        ospill = dram.tile([P.HG, 128, P.T], F32R)
        rms_dram = dram.tile([1, P.T], F32)

        # ---------------- phase A: rms + c_kv ----------------
        with tc.tile_pool(name="pa", bufs=1) as pa, \
             tc.tile_pool(name="pa2", bufs=2) as pa2, \
             tc.tile_pool(name="psA", bufs=1, space="PSUM") as psA:
            hub_sb = pa.tile([128, P.HC, P.T], F32R)
            wdt_sb = pa.tile([128, P.HC, P.DLAT], F32R)
            for hc in range(P.HC):
                nc.sync.dma_start(out=hub_sb[:, hc, :],
                                  in_=ins["hubt"][hc * 128:(hc + 1) * 128, :].bitcast(F32R))
                nc.sync.dma_start(out=wdt_sb[:, hc, :],
                                  in_=ins["wdt"][hc * 128:(hc + 1) * 128, :].bitcast(F32R))

            ssq = [psA.tile([1, 512], F32, name=f"ssq{ts}", tag=f"ssq{ts}")
                   for ts in range(P.T4)]
            for hc in range(P.HC):
                for ts in range(P.T4):
                    sq = pa2.tile([128, 512], F32R, tag="sq")
                    nc.vector.tensor_tensor(sq[:], hub_sb[:, hc, ts * 512:(ts + 1) * 512],
                                            hub_sb[:, hc, ts * 512:(ts + 1) * 512], MUL)
                    nc.tensor.matmul(ssq[ts][:], ones_sb[:], sq[:],
                                     start=(hc == 0), stop=(hc == P.HC - 1))
            sqrt_row = pa.tile([1, P.T], F32)
            for ts in range(P.T4):
                nc.scalar.activation(sqrt_row[:, ts * 512:(ts + 1) * 512], ssq[ts][:],
                                     SQRT, bias=eps_sb[:], scale=1.0 / P.DHUB)
            nc.vector.reciprocal_approx_fast(out=sqrt_row[:], in_=sqrt_row[:])
            nc.sync.dma_start(out=rms_dram[:], in_=sqrt_row[:])
            rms_b = pa.tile([128, P.T], F32)
            nc.sync.dma_start(out=rms_b[:], in_=_bcast_ap(rms_dram[:]))

            for lc in range(P.LC):
                for tcol in range(P.T4):
                    ckvp = psA.tile([128, 512], F32, tag="ckvp", bufs=2)
                    for hc in range(P.HC):
                        nc.tensor.matmul(
                            ckvp[:],
                            wdt_sb[:, hc, lc * 128:(lc + 1) * 128],
                            hub_sb[:, hc, tcol * 512:(tcol + 1) * 512],
                            start=(hc == 0), stop=(hc == P.HC - 1))
                    nc.vector.tensor_tensor(
                        ckv[:, lc, tcol * 512:(tcol + 1) * 512],
                        ckvp[:], rms_b[:, tcol * 512:(tcol + 1) * 512], MUL)

        # ---------------- phase B: q projection + rope + spill ----------------
        with tc.tile_pool(name="pb", bufs=1) as pb, \
             tc.tile_pool(name="pb2", bufs=2) as pb2, \
             tc.tile_pool(name="psB", bufs=2, space="PSUM") as psB:
            for th in range(P.TH):
                t0 = th * P.THW
                xt_half = pb.tile([128, P.DC, P.THW], F32R, tag="xt_half")
                for dc in range(P.DC):
                    nc.sync.dma_start(
                        out=xt_half[:, dc, :],
                        in_=ins["xt"][dc * 128:(dc + 1) * 128, t0:t0 + P.THW].bitcast(F32R))
                for h in range(P.HG):
                    wq_h = pb2.tile([128, P.DC, DH], F32R, tag="wq_h")
                    for dc in range(P.DC):
                        nc.sync.dma_start(
                            out=wq_h[:, dc, :],
                            in_=ins["wqt"][dc * 128:(dc + 1) * 128,
                                           h * DH:(h + 1) * DH].bitcast(F32R))
                    for t4 in range(P.THW // 512):
                        tq0 = t0 + t4 * 512
                        qp = psB.tile([128, 512], F32, tag="qp")
                        for dc in range(P.DC):
                            nc.tensor.matmul(qp[:], wq_h[:, dc, :],
                                             xt_half[:, dc, t4 * 512:(t4 + 1) * 512],
                                             start=(dc == 0), stop=(dc == P.DC - 1))
                        qc = pb2.tile([128, 512], F32, tag="qc")
                        nc.vector.tensor_tensor(qc[:], qp[:],
                                                rope_c[:, tq0:tq0 + 512], MUL)
                        qs = pb2.tile([128, 512], F32R, tag="qs")
                        nc.vector.tensor_tensor(qs[:], qp[:],
                                                rope_sp[:, tq0:tq0 + 512], MUL)
                        qw = psB.tile([128, 512], F32, tag="qw")
                        nc.tensor.matmul(qw[:], perm_sb[:], qs[:],
                                         start=True, stop=True)
                        qr = pb2.tile([128, 512], F32R, tag="qr")
                        nc.vector.tensor_tensor(qr[:], qc[:], qw[:], ADD)
                        nc.sync.dma_start(out=qspill[h, :, tq0:tq0 + 512], in_=qr[:])

        # ---------------- phase C/D: v, per-head k + attention ----------------
        with tc.tile_pool(name="pc", bufs=1) as pc, \
             tc.tile_pool(name="pd2", bufs=2) as pd2, \
             tc.tile_pool(name="pd3", bufs=3) as pd3:
            wut_k_sb = pc.tile([128, P.LC, P.GD], F32R)
            for lc in range(P.LC):
                nc.sync.dma_start(out=wut_k_sb[:, lc, :],
                                  in_=ins["wut_k"][lc * 128:(lc + 1) * 128, :].bitcast(F32R))
            v_all = pc.tile([128, P.ST, P.GD], F32R)
            ncol = (P.GD + 511) // 512
            cw = min(512, P.GD)
            with tc.tile_pool(name="pcv", bufs=1) as pcv, \
                 tc.tile_pool(name="psC", bufs=2, space="PSUM") as psC:
                wut_v_sb = pcv.tile([128, P.LC, P.GD], F32R)
                for lc in range(P.LC):
                    nc.sync.dma_start(out=wut_v_sb[:, lc, :],
                                      in_=ins["wut_v"][lc * 128:(lc + 1) * 128, :].bitcast(F32R))
                for st in range(P.ST):
                    vps = psC.tile([128, P.GD], F32, tag="vps")
                    for lc in range(P.LC):
                        for hq in range(ncol):
                            nc.tensor.matmul(
                                vps[:, hq * cw:(hq + 1) * cw],
                                ckv[:, lc, st * 128:(st + 1) * 128],
                                wut_v_sb[:, lc, hq * cw:(hq + 1) * cw],
                                start=(lc == 0), stop=(lc == P.LC - 1))
                    nc.scalar.copy(out=v_all[:, st, :], in_=vps[:])


            with tc.tile_pool(name="psD", bufs=2, space="PSUM") as psD:
                for h in range(P.HG):
                    # k projection + rope
                    kT = pd2.tile([128, P.T], F32R, tag="kT", bufs=1)
                    for s4 in range(P.T4):
                        ks0 = s4 * 512
                        kps = psD.tile([128, 512], F32, tag="kps")
                        for lc in range(P.LC):
                            nc.tensor.matmul(kps[:],
                                             wut_k_sb[:, lc, h * DH:(h + 1) * DH],
                                             ckv[:, lc, ks0:ks0 + 512],
                                             start=(lc == 0), stop=(lc == P.LC - 1))
                        kc = pd2.tile([128, 512], F32, tag="kc")
                        nc.vector.tensor_tensor(kc[:], kps[:],
                                                rope_c[:, ks0:ks0 + 512], MUL)
                        ks = pd2.tile([128, 512], F32R, tag="ks")
                        nc.vector.tensor_tensor(ks[:], kps[:],
                                                rope_sp[:, ks0:ks0 + 512], MUL)
                        kw = psD.tile([128, 512], F32, tag="kps")
                        nc.tensor.matmul(kw[:], perm_sb[:], ks[:],
                                         start=True, stop=True)
                        nc.vector.tensor_tensor(kT[:, ks0:ks0 + 512], kc[:], kw[:], ADD)

                    qT = pd2.tile([128, P.T], F32R, tag="qT", bufs=1)
                    nc.sync.dma_start(out=qT[:], in_=qspill[h])

                    oT = pd2.tile([128, P.T], F32R, tag="oT", bufs=1)
                    for g in range(P.NG):
                        g0 = g * TQ
                        ops = psD.tile([128, TQ], F32, tag="ops")
                        denp = psD.tile([1, TQ], F32, tag="denp")
                        nchunk = (g + 1) * (TQ // 128)
                        for c in range(nchunk):
                            scp = psD.tile([128, TQ], F32, tag="scp")
                            nc.tensor.matmul(scp[:],
                                             kT[:, c * 128:(c + 1) * 128],
                                             qT[:, g0:g0 + TQ],
                                             start=True, stop=True)
                            pT = pd3.tile([128, TQ], F32R, tag="pT")
                            nc.scalar.activation(pT[:], scp[:], EXP, scale=scale)
                            r = c - (g0 // 128)
                            if r >= 0:
                                nc.vector.tensor_tensor(pT[:], pT[:], masks_t[r][:], MUL)
                            nc.tensor.matmul(denp[:], ones_sb[:], pT[:],
                                             start=(c == 0), stop=(c == nchunk - 1))
                            nc.tensor.matmul(ops[:],
                                             v_all[:, c, h * DH:(h + 1) * DH], pT[:],
                                             start=(c == 0), stop=(c == nchunk - 1))
                        den_sb = pd2.tile([1, TQ], F32, tag="den_sb")
                        nc.scalar.copy(out=den_sb[:], in_=denp[:])
                        den_r = pd2.tile([1, TQ], F32, tag="den_r")
                        nc.vector.reciprocal_approx_fast(out=den_r[:], in_=den_sb[:])
                        den_dram = dram.tile([1, TQ], F32, tag="den_dram", bufs=4)
                        nc.sync.dma_start(out=den_dram[:], in_=den_r[:])
                        den_b = pd2.tile([128, TQ], F32, tag="den_b")
                        nc.sync.dma_start(out=den_b[:], in_=_bcast_ap(den_dram[:]))
                        nc.vector.tensor_tensor(oT[:, g0:g0 + TQ], ops[:], den_b[:], MUL)
                    nc.sync.dma_start(out=ospill[h], in_=oT[:])

        # ---------------- phase E: output projection ----------------
        with tc.tile_pool(name="pe", bufs=1) as pe, \
             tc.tile_pool(name="pe2", bufs=2) as pe2, \
             tc.tile_pool(name="psE", bufs=2, space="PSUM") as psE:
            wot_sb = pe.tile([128, P.HG, P.D], F32R)
            for h in range(P.HG):
                nc.sync.dma_start(out=wot_sb[:, h, :],
                                  in_=ins["wot"][h * 128:(h + 1) * 128, :].bitcast(F32R))
            for tt in range(P.ST):
                oth = pe2.tile([128, P.HG, DH], F32R, tag="oth")
                for h in range(P.HG):
                    nc.sync.dma_start(out=oth[:, h, :],
                                      in_=ospill[h, :, tt * 128:(tt + 1) * 128])
                out_sb = pe2.tile([128, P.D], F32, tag="out_sb")
                for ec in range(P.EC):
                    outp = psE.tile([128, 512], F32, tag="outp")
                    for h in range(P.HG):
                        nc.tensor.matmul(outp[:], oth[:, h, :],
                                         wot_sb[:, h, ec * 512:(ec + 1) * 512],
                                         start=(h == 0), stop=(h == P.HG - 1))
                    nc.scalar.copy(out=out_sb[:, ec * 512:(ec + 1) * 512], in_=outp[:])
                nc.sync.dma_start(out=out_p[tt * 128:(tt + 1) * 128, :], in_=out_sb[:])


def make_rope_tables(cos: np.ndarray, sin: np.ndarray):
    """cos/sin [T, DH/2] -> C, S' [128, T] (fp32)."""
    Tn = cos.shape[0]
    C = np.repeat(np.ascontiguousarray(cos.T), 2, axis=0).astype(np.float32)
    SP = np.empty((DH, Tn), np.float32)
    SP[0::2] = sin.T
    SP[1::2] = -sin.T
    return C, SP


def make_masks():
    m = np.zeros((4, 128, TQ), np.float32)
    for r in range(4):
        for j in range(TQ):
            ti = j // 128
            if ti > r:
                m[r, :, j] = 1.0
            elif ti == r:
                m[r, :j % 128 + 1, j] = 1.0
    return m


def make_core_inputs(H_q, hub_content, Wq, norm_w, W_down, W_up, Wo, cos, sin):
    """Full inputs -> per-core in_maps (list of 8 dicts)."""
    C, SP = make_rope_tables(np.asarray(cos), np.asarray(sin))
    masks = make_masks()
    ones_w = np.ones((128, 1), np.float32)
    perm = np.zeros((128, 128), np.float32)
    for i in range(0, 128, 2):
        perm[i, i + 1] = 1.0
        perm[i + 1, i] = 1.0
    wdt = round_tf32((np.asarray(W_down) * np.asarray(norm_w)[None, :]).T)
    wut = np.asarray(W_up).T  # [D_LAT, 2D]
    in_maps = []
    for b in range(B):
        xt = round_tf32(np.asarray(H_q[b]).T)
        hubt = round_tf32(np.asarray(hub_content[b]).T)
        for g in range(G):
            gch = slice(g * HG * DH, (g + 1) * HG * DH)
            vch = slice(D + g * HG * DH, D + (g + 1) * HG * DH)
            in_maps.append({
                "xt": xt,
                "hubt": hubt,
                "wqt": round_tf32(np.asarray(Wq)[gch, :].T),
                "wdt": wdt,
                "wut_k": round_tf32(wut[:, gch]),
                "wut_v": round_tf32(wut[:, vch]),
                "wot": round_tf32(np.asarray(Wo)[:, gch].T),
                "rope_c": C,
                "rope_sp": SP,
                "masks": masks,
                "ones_w": ones_w,
                "perm": perm,
            })
    return in_maps


_NC_CACHE = {}


def _get_nc():
    if "nc" in _NC_CACHE:
        return _NC_CACHE["nc"]
    P = Cfg()
    nc = bacc.Bacc(None, target_bir_lowering=False)
    ins = {}
    for name, (shape, dt_) in input_specs(P).items():
        ins[name] = nc.dram_tensor(name, shape, dt_, kind="ExternalInput")[:]
    outs = {"out_p": nc.dram_tensor("out_p", [P.T, P.D], F32, kind="ExternalOutput")[:]}
    with tile.TileContext(nc) as tc:
        build_kernel(tc, outs, ins, P)
    nc.compile()
    _NC_CACHE["nc"] = nc
    return nc


def kernel(H_q, hub_content, Wq, norm_w, W_down, W_up, Wo, cos, sin):
    nc = _get_nc()
    in_maps = make_core_inputs(H_q, hub_content, Wq, norm_w, W_down, W_up,
                               Wo, cos, sin)
    res = run_bass_kernel_spmd(nc, in_maps, core_ids=list(range(NCORES)))
    out = np.empty((B, T, D), np.float32)
    for b in range(B):
        out[b] = res.results[2 * b]["out_p"] + res.results[2 * b + 1]["out_p"]
    return out


# revision 10
# speedup vs baseline: 154.8222x; 154.8222x over previous
"""CWT latent attention kernel for 8 Trainium2 NeuronCores.

Sharding: core c = 2*b + g handles batch b and head-group g (8 of 16 heads).
Each core computes its heads' q/k/v, causal attention, and a partial output
projection (contracted over its heads' channels); the host sums the two
partials per batch.

Device layout notes:
- All matmul operands live "transposed" (contraction dim on partitions);
  activations are fed pre-transposed from the host, so no on-device
  transposes are needed anywhere.
- Attention computes scoresT = K^T-tile.T @ Q (shape [s, tq]); softmax skips
  the max-subtraction (scores are O(10), exp cannot overflow in fp32), the
  causal mask is a 0/1 multiply on diagonal tiles, and the denominator is a
  ones-vector matmul accumulated alongside the A@V matmul. The division is
  applied to the attention output tiles via a partition-broadcast of 1/den.
- RoPE is applied in [dh, t] layout as rot = x*C + swap(x*S'), where C/S'
  are host-precomputed [128, T] tables and swap exchanges adjacent partition
  pairs via a PE matmul with a pair-swap permutation matrix.
- Matmuls run in float32r (TF32); inputs are TF32-rounded on the host, and
  every on-device tensor feeding a matmul is written with dtype float32r.
"""

import math
from dataclasses import dataclass

import numpy as np

import concourse.bass as bass
import concourse.mybir as mybir
import concourse.tile as tile
from concourse import bacc
from concourse.bass_utils import run_bass_kernel_spmd

F32 = mybir.dt.float32
F32R = mybir.dt.float32r
EXP = mybir.ActivationFunctionType.Exp
SQRT = mybir.ActivationFunctionType.Sqrt
MUL = mybir.AluOpType.mult
ADD = mybir.AluOpType.add

# problem constants
B, T, D = 4, 2048, 2048
H, DH = 16, 128
D_LAT, D_HUB = 512, 1024
EPS = 1e-6
G = 2               # head groups == cores per batch
HG = H // G         # heads per core
NCORES = 8
TQ = 512            # tq group width for attention


@dataclass
class Cfg:
    T: int = T
    D: int = D
    DHUB: int = D_HUB
    DLAT: int = D_LAT
    HG: int = HG
    TH: int = 2      # t-halves for XT residency in the q-projection

    @property
    def DC(self):  return self.D // 128        # xt chunks
    @property
    def HC(self):  return self.DHUB // 128     # hub chunks
    @property
    def LC(self):  return self.DLAT // 128     # latent chunks
    @property
    def ST(self):  return self.T // 128        # s tiles
    @property
    def NG(self):  return self.T // TQ         # tq groups
    @property
    def T4(self):  return self.T // 512        # 512-wide column chunks
    @property
    def THW(self): return self.T // self.TH    # t-half width
    @property
    def GD(self):  return self.HG * DH         # group channel width
    @property
    def EC(self):  return self.D // 512        # output e columns


def round_tf32(x: np.ndarray) -> np.ndarray:
    x = np.ascontiguousarray(x, np.float32)
    u = x.view(np.uint32)
    r = (u + 0x1000 + ((u >> 13) & 1)) & np.uint32(0xFFFFE000)
    return r.view(np.float32)


def input_specs(P: Cfg):
    return {
        "xt":      ([P.D, P.T], F32),
        "hubt":    ([P.DHUB, P.T], F32),
        "wqt":     ([P.D, P.GD], F32),
        "wdt":     ([P.DHUB, P.DLAT], F32),
        "wut_k":   ([P.DLAT, P.GD], F32),
        "wut_v":   ([P.DLAT, P.GD], F32),
        "wot":     ([P.GD, P.D], F32),
        "rope_c":  ([128, P.T], F32),
        "rope_sp": ([128, P.T], F32),
        "masks":   ([4, 128, TQ], F32),
        "ones_w":  ([128, 1], F32),
        "perm":    ([128, 128], F32),
    }


def _bcast_ap(row: bass.AP, p: int = 128) -> bass.AP:
    """[1, N] DRAM row -> partition-broadcast [p, N] read AP."""
    return bass.AP(tensor=row.tensor, offset=row.offset,
                   ap=[[0, p]] + [list(d) for d in row.ap[1:]])


def build_kernel(tc: tile.TileContext, outs: dict, ins: dict, P: Cfg):
    nc = tc.nc
    scale = 1.0 / math.sqrt(DH)
    out_p = outs["out_p"]

    with tc.tile_pool(name="tables", bufs=1) as tables, \
         tc.tile_pool(name="dram", bufs=1, space="DRAM") as dram, \
         tc.tile_pool(name="ckvpool", bufs=1) as ckvpool:

        rope_c = tables.tile([128, P.T], F32)
        nc.sync.dma_start(out=rope_c, in_=ins["rope_c"][:])
        rope_sp = tables.tile([128, P.T], F32)
        nc.sync.dma_start(out=rope_sp, in_=ins["rope_sp"][:])
        # masks dram is [4,128,TQ]; load each mask as a [128, TQ] tile
        masks_t = [tables.tile([128, TQ], mybir.dt.bfloat16, name=f"mask{r}",
                               tag=f"mask{r}") for r in range(4)]
        for r in range(4):
            nc.gpsimd.dma_start(out=masks_t[r], in_=ins["masks"][r])
        ones_sb = tables.tile([128, 1], F32R)
        nc.sync.dma_start(out=ones_sb, in_=ins["ones_w"][:].bitcast(F32R))
        eps_sb = tables.tile([1, 1], F32)
        nc.vector.memset(eps_sb, EPS)
        perm_sb = tables.tile([128, 128], F32R)
        nc.sync.dma_start(out=perm_sb, in_=ins["perm"][:].bitcast(F32R))

        ckv = ckvpool.tile([128, P.LC, P.T], F32R)

        qspill = dram.tile([P.HG, 128, P.T], F32R)
        ospill = dram.tile([P.HG, 128, P.T], F32R)
        rms_dram = dram.tile([1, P.T], F32)

        # ---------------- phase A: rms + c_kv ----------------
        with tc.tile_pool(name="pa", bufs=1) as pa, \
             tc.tile_pool(name="pa2", bufs=2) as pa2, \
             tc.tile_pool(name="psA", bufs=1, space="PSUM") as psA:
            hub_sb = pa.tile([128, P.HC, P.T], F32R)
            wdt_sb = pa.tile([128, P.HC, P.DLAT], F32R)
            for hc in range(P.HC):
                nc.sync.dma_start(out=hub_sb[:, hc, :],
                                  in_=ins["hubt"][hc * 128:(hc + 1) * 128, :].bitcast(F32R))
                nc.sync.dma_start(out=wdt_sb[:, hc, :],
                                  in_=ins["wdt"][hc * 128:(hc + 1) * 128, :].bitcast(F32R))

            ssq = [psA.tile([1, 512], F32, name=f"ssq{ts}", tag=f"ssq{ts}")
                   for ts in range(P.T4)]
            for hc in range(P.HC):
                for ts in range(P.T4):
                    sq = pa2.tile([128, 512], F32R, tag="sq")
                    nc.vector.tensor_tensor(sq[:], hub_sb[:, hc, ts * 512:(ts + 1) * 512],
                                            hub_sb[:, hc, ts * 512:(ts + 1) * 512], MUL)
                    nc.tensor.matmul(ssq[ts][:], ones_sb[:], sq[:],
                                     start=(hc == 0), stop=(hc == P.HC - 1))
            sqrt_row = pa.tile([1, P.T], F32)
            for ts in range(P.T4):
                nc.scalar.activation(sqrt_row[:, ts * 512:(ts + 1) * 512], ssq[ts][:],
                                     SQRT, bias=eps_sb[:], scale=1.0 / P.DHUB)
            nc.vector.reciprocal_approx_fast(out=sqrt_row[:], in_=sqrt_row[:])
            nc.sync.dma_start(out=rms_dram[:], in_=sqrt_row[:])
            rms_b = pa.tile([128, P.T], F32)
            nc.sync.dma_start(out=rms_b[:], in_=_bcast_ap(rms_dram[:]))

            for lc in range(P.LC):
                for tcol in range(P.T4):
                    ckvp = psA.tile([128, 512], F32, tag="ckvp", bufs=2)
                    for hc in range(P.HC):
                        nc.tensor.matmul(
                            ckvp[:],
                            wdt_sb[:, hc, lc * 128:(lc + 1) * 128],
                            hub_sb[:, hc, tcol * 512:(tcol + 1) * 512],
                            start=(hc == 0), stop=(hc == P.HC - 1))
                    nc.vector.tensor_tensor(
                        ckv[:, lc, tcol * 512:(tcol + 1) * 512],
                        ckvp[:], rms_b[:, tcol * 512:(tcol + 1) * 512], MUL)

        # ---------------- phase B: q projection + rope + spill ----------------
        with tc.tile_pool(name="pb", bufs=1) as pb, \
             tc.tile_pool(name="pb2", bufs=2) as pb2, \
             tc.tile_pool(name="psB", bufs=2, space="PSUM") as psB:
            for th in range(P.TH):
                t0 = th * P.THW
                xt_half = pb.tile([128, P.DC, P.THW], F32R, tag="xt_half")
                for dc in range(P.DC):
                    nc.sync.dma_start(
                        out=xt_half[:, dc, :],
                        in_=ins["xt"][dc * 128:(dc + 1) * 128, t0:t0 + P.THW].bitcast(F32R))
                for h in range(P.HG):
                    wq_h = pb2.tile([128, P.DC, DH], F32R, tag="wq_h")
                    for dc in range(P.DC):
                        nc.sync.dma_start(
                            out=wq_h[:, dc, :],
                            in_=ins["wqt"][dc * 128:(dc + 1) * 128,
                                           h * DH:(h + 1) * DH].bitcast(F32R))
                    for t4 in range(P.THW // 512):
                        tq0 = t0 + t4 * 512
                        qp = psB.tile([128, 512], F32, tag="qp")
                        for dc in range(P.DC):
                            nc.tensor.matmul(qp[:], wq_h[:, dc, :],
                                             xt_half[:, dc, t4 * 512:(t4 + 1) * 512],
                                             start=(dc == 0), stop=(dc == P.DC - 1))
                        qc = pb2.tile([128, 512], F32, tag="qc")
                        nc.vector.tensor_tensor(qc[:], qp[:],
                                                rope_c[:, tq0:tq0 + 512], MUL)
                        qs = pb2.tile([128, 512], F32R, tag="qs")
                        nc.vector.tensor_tensor(qs[:], qp[:],
                                                rope_sp[:, tq0:tq0 + 512], MUL)
                        qw = psB.tile([128, 512], F32, tag="qw")
                        nc.tensor.matmul(qw[:], perm_sb[:], qs[:],
                                         start=True, stop=True)
                        qr = pb2.tile([128, 512], F32R, tag="qr")
                        nc.vector.tensor_tensor(qr[:], qc[:], qw[:], ADD)
                        nc.sync.dma_start(out=qspill[h, :, tq0:tq0 + 512], in_=qr[:])

        # ---------------- phase C/D: v, per-head k + attention ----------------
        with tc.tile_pool(name="pc", bufs=1) as pc, \
             tc.tile_pool(name="pd2", bufs=2) as pd2, \
             tc.tile_pool(name="pd3", bufs=3) as pd3:
            v_all = pc.tile([128, P.ST, P.GD], F32R)
            ncol = (P.GD + 511) // 512
            cw = min(512, P.GD)
            with tc.tile_pool(name="pcv", bufs=1) as pcv, \
                 tc.tile_pool(name="psC", bufs=2, space="PSUM") as psC:
                wut_v_sb = pcv.tile([128, P.LC, P.GD], F32R)
                for lc in range(P.LC):
                    nc.sync.dma_start(out=wut_v_sb[:, lc, :],
                                      in_=ins["wut_v"][lc * 128:(lc + 1) * 128, :].bitcast(F32R))
                for st in range(P.ST):
                    vps = psC.tile([128, P.GD], F32, tag="vps")
                    for lc in range(P.LC):
                        for hq in range(ncol):
                            nc.tensor.matmul(
                                vps[:, hq * cw:(hq + 1) * cw],
                                ckv[:, lc, st * 128:(st + 1) * 128],
                                wut_v_sb[:, lc, hq * cw:(hq + 1) * cw],
                                start=(lc == 0), stop=(lc == P.LC - 1))
                    nc.scalar.copy(out=v_all[:, st, :], in_=vps[:])


            with tc.tile_pool(name="psD", bufs=2, space="PSUM") as psD:
                for h in range(P.HG):
                    # k projection + rope (per-head slice of W_up^T k-columns)
                    wk_h = pd2.tile([128, P.LC, DH], F32R, tag="wk_h")
                    for lc in range(P.LC):
                        nc.sync.dma_start(
                            out=wk_h[:, lc, :],
                            in_=ins["wut_k"][lc * 128:(lc + 1) * 128,
                                             h * DH:(h + 1) * DH].bitcast(F32R))
                    kT = pd2.tile([128, P.T], F32R, tag="kT", bufs=2)
                    for s4 in range(P.T4):
                        ks0 = s4 * 512
                        kps = psD.tile([128, 512], F32, tag="kps")
                        for lc in range(P.LC):
                            nc.tensor.matmul(kps[:],
                                             wk_h[:, lc, :],
                                             ckv[:, lc, ks0:ks0 + 512],
                                             start=(lc == 0), stop=(lc == P.LC - 1))
                        kc = pd2.tile([128, 512], F32, tag="kc")
                        nc.vector.tensor_tensor(kc[:], kps[:],
                                                rope_c[:, ks0:ks0 + 512], MUL)
                        ks = pd2.tile([128, 512], F32R, tag="ks")
                        nc.vector.tensor_tensor(ks[:], kps[:],
                                                rope_sp[:, ks0:ks0 + 512], MUL)
                        kw = psD.tile([128, 512], F32, tag="kps")
                        nc.tensor.matmul(kw[:], perm_sb[:], ks[:],
                                         start=True, stop=True)
                        nc.vector.tensor_tensor(kT[:, ks0:ks0 + 512], kc[:], kw[:], ADD)

                    qT = pd2.tile([128, P.T], F32R, tag="qT", bufs=2)
                    nc.sync.dma_start(out=qT[:], in_=qspill[h])

                    oT = pd2.tile([128, P.T], F32R, tag="oT", bufs=1)
                    for g in range(P.NG):
                        g0 = g * TQ
                        ops = psD.tile([128, TQ], F32, tag="ops")
                        denp = psD.tile([1, TQ], F32, tag="denp")
                        nchunk = (g + 1) * (TQ // 128)
                        for c in range(nchunk):
                            scp = psD.tile([128, TQ], F32, tag="scp")
                            nc.tensor.matmul(scp[:],
                                             kT[:, c * 128:(c + 1) * 128],
                                             qT[:, g0:g0 + TQ],
                                             start=True, stop=True)
                            pT = pd3.tile([128, TQ], F32R, tag="pT")
                            nc.scalar.activation(pT[:], scp[:], EXP, scale=scale)
                            r = c - (g0 // 128)
                            if r >= 0:
                                nc.vector.tensor_tensor(pT[:], pT[:], masks_t[r][:], MUL)
                            nc.tensor.matmul(denp[:], ones_sb[:], pT[:],
                                             start=(c == 0), stop=(c == nchunk - 1))
                            nc.tensor.matmul(ops[:],
                                             v_all[:, c, h * DH:(h + 1) * DH], pT[:],
                                             start=(c == 0), stop=(c == nchunk - 1))
                        den_sb = pd2.tile([1, TQ], F32, tag="den_sb")
                        nc.scalar.copy(out=den_sb[:], in_=denp[:])
                        den_r = pd2.tile([1, TQ], F32, tag="den_r")
                        nc.vector.reciprocal_approx_fast(out=den_r[:], in_=den_sb[:])
                        den_dram = dram.tile([1, TQ], F32, tag="den_dram", bufs=4)
                        nc.sync.dma_start(out=den_dram[:], in_=den_r[:])
                        den_b = pd2.tile([128, TQ], F32, tag="den_b")
                        nc.sync.dma_start(out=den_b[:], in_=_bcast_ap(den_dram[:]))
                        nc.vector.tensor_tensor(oT[:, g0:g0 + TQ], ops[:], den_b[:], MUL)
                    nc.sync.dma_start(out=ospill[h], in_=oT[:])

        # ---------------- phase E: output projection ----------------
        with tc.tile_pool(name="pe", bufs=1) as pe, \
             tc.tile_pool(name="pe2", bufs=2) as pe2, \
             tc.tile_pool(name="psE", bufs=2, space="PSUM") as psE:
            wot_sb = pe.tile([128, P.HG, P.D], F32R)
            for h in range(P.HG):
                nc.sync.dma_start(out=wot_sb[:, h, :],
                                  in_=ins["wot"][h * 128:(h + 1) * 128, :].bitcast(F32R))
            for tt in range(P.ST):
                oth = pe2.tile([128, P.HG, DH], F32R, tag="oth")
                for h in range(P.HG):
                    nc.sync.dma_start(out=oth[:, h, :],
                                      in_=ospill[h, :, tt * 128:(tt + 1) * 128])
                out_sb = pe2.tile([128, P.D], F32, tag="out_sb")
                for ec in range(P.EC):
                    outp = psE.tile([128, 512], F32, tag="outp")
                    for h in range(P.HG):
                        nc.tensor.matmul(outp[:], oth[:, h, :],
                                         wot_sb[:, h, ec * 512:(ec + 1) * 512],
                                         start=(h == 0), stop=(h == P.HG - 1))
                    nc.scalar.copy(out=out_sb[:, ec * 512:(ec + 1) * 512], in_=outp[:])
                nc.sync.dma_start(out=out_p[tt * 128:(tt + 1) * 128, :], in_=out_sb[:])


def make_rope_tables(cos: np.ndarray, sin: np.ndarray):
    """cos/sin [T, DH/2] -> C, S' [128, T] (fp32)."""
    Tn = cos.shape[0]
    C = np.repeat(np.ascontiguousarray(cos.T), 2, axis=0).astype(np.float32)
    SP = np.empty((DH, Tn), np.float32)
    SP[0::2] = sin.T
    SP[1::2] = -sin.T
    return C, SP


def make_masks():
    m = np.zeros((4, 128, TQ), np.float32)
    for r in range(4):
        for j in range(TQ):
            ti = j // 128
            if ti > r:
                m[r, :, j] = 1.0
            elif ti == r:
                m[r, :j % 128 + 1, j] = 1.0
    return m


def make_core_inputs(H_q, hub_content, Wq, norm_w, W_down, W_up, Wo, cos, sin):
    """Full inputs -> per-core in_maps (list of 8 dicts)."""
    C, SP = make_rope_tables(np.asarray(cos), np.asarray(sin))
    masks = make_masks()
    ones_w = np.ones((128, 1), np.float32)
    perm = np.zeros((128, 128), np.float32)
    for i in range(0, 128, 2):
        perm[i, i + 1] = 1.0
        perm[i + 1, i] = 1.0
    wdt = round_tf32((np.asarray(W_down) * np.asarray(norm_w)[None, :]).T)
    wut = np.asarray(W_up).T  # [D_LAT, 2D]
    in_maps = []
    for b in range(B):
        xt = round_tf32(np.asarray(H_q[b]).T)
        hubt = round_tf32(np.asarray(hub_content[b]).T)
        for g in range(G):
            gch = slice(g * HG * DH, (g + 1) * HG * DH)
            vch = slice(D + g * HG * DH, D + (g + 1) * HG * DH)
            in_maps.append({
                "xt": xt,
                "hubt": hubt,
                "wqt": round_tf32(np.asarray(Wq)[gch, :].T),
                "wdt": wdt,
                "wut_k": round_tf32(wut[:, gch]),
                "wut_v": round_tf32(wut[:, vch]),
                "wot": round_tf32(np.asarray(Wo)[:, gch].T),
                "rope_c": C,
                "rope_sp": SP,
                "masks": masks,
                "ones_w": ones_w,
                "perm": perm,
            })
    return in_maps


_NC_CACHE = {}


def _get_nc():
    if "nc" in _NC_CACHE:
        return _NC_CACHE["nc"]
    P = Cfg()
    nc = bacc.Bacc(None, target_bir_lowering=False)
    ins = {}
    for name, (shape, dt_) in input_specs(P).items():
        ins[name] = nc.dram_tensor(name, shape, dt_, kind="ExternalInput")[:]
    outs = {"out_p": nc.dram_tensor("out_p", [P.T, P.D], F32, kind="ExternalOutput")[:]}
    with tile.TileContext(nc) as tc:
        build_kernel(tc, outs, ins, P)
    nc.compile()
    _NC_CACHE["nc"] = nc
    return nc


def kernel(H_q, hub_content, Wq, norm_w, W_down, W_up, Wo, cos, sin):
    nc = _get_nc()
    in_maps = make_core_inputs(H_q, hub_content, Wq, norm_w, W_down, W_up,
                               Wo, cos, sin)
    res = run_bass_kernel_spmd(nc, in_maps, core_ids=list(range(NCORES)))
    out = np.empty((B, T, D), np.float32)
    for b in range(B):
        out[b] = res.results[2 * b]["out_p"] + res.results[2 * b + 1]["out_p"]
    return out
